# revision 11
# baseline (speedup 1.0000x reference)
"""CASVDDenseMul fused kernel for 8 Trainium2 NeuronCores.

Reference computation (fp32):
    chi = sigmoid(context @ W + B)          # [B, R]
    t   = (inputs @ U) * (S * chi)          # [B, R]
    out = relu(t @ V.T + 2*bias)            # [B, UNITS]

Sharding: data-parallel over batch; each of the 8 cores handles 512 rows.
All factor weights (U, S, V, W, B, bias) are replicated.

The kernel is HBM-bandwidth/PE co-limited (~349 GB/s effective per core;
the core clock drops to ~5/6 nominal under sustained load). Precision per
operand is chosen to fit the 2e-2 rel-err budget with margin (measured
1.34e-2 end to end, dominated by x):
  - x rides as fp8 e3m4 (values ~N(0,1) fit its range natively); the PE
    consumes mixed fp16-stationary x fp8-moving matmuls at 1 row/cycle.
  - U, W, context, V, t', and the output are fp16; accumulation is fp32.
Total HBM traffic: ~11.3 MB/core vs 26.8 MB for the fp32 baseline.

Scheduling (from trace analysis):
  - One DMA ring saturates HBM by itself and transfers within a ring run
    strictly in order, so ALL bulk traffic rides the sync ring in the
    exact order the bytes are needed: (wctx chunk | U quad | x quad)
    interleaved, then vt groups, then out pairs.
  - chi's 8 matmuls are laced chunk-by-chunk into the first four mm1
    quads (the PE is the critical resource end to end; running chi
    before mm1 would push everything out by ~2us). The sigmoid rides the
    otherwise-empty scalar queue, the S-multiply the vector queue.
  - mm2 PSUM tiles are single banks rotating through a 6-buffer pool;
    with fewer buffers the matmul<->evacuation ping-pong serializes the
    output phase.
  - PSUM evacuation is the output-phase engine constraint (~0.7-0.9us
    per 512-col tile on either ACT or DVE; GPSIMD has no PSUM port), so
    ACT takes even unit-tiles (relu+bias fused via its per-partition
    bias port on the transposed [units, batch] layout) and DVE odd ones;
    each evacuated pair shares one out DMA of 2 KB/partition rows.
"""

import numpy as np

from concourse import bacc, mybir
from concourse import tile
from concourse.bass_utils import run_bass_kernel_spmd

N_CORES = 8
B_SZ, N_IN, N_CTX, UNITS, RANK = 4096, 4096, 512, 4096, 256
BS = B_SZ // N_CORES  # 512 batch rows per core

P = 128
KC_CTX = N_CTX // P      # 4 contraction chunks for ctx @ W
RT = RANK // P           # 2 rank tiles
MT = UNITS // P          # 32 unit (output) tiles
QUADS = N_IN // (4 * P)  # 8 packed x/U chunk quads
MG = 8                   # vt groups (512 units each)
MW = UNITS // MG         # 512
OG = MT // 2             # 16 output tile pairs
UXW = RANK + BS          # 768 packed columns per k-chunk

FP32 = mybir.dt.float32
FP16 = mybir.dt.float16
FP8 = mybir.dt.float8e3


def _build_nc():
    nc = bacc.Bacc("TRN2", target_bir_lowering=False, debug=False, enable_asserts=False)

    uq = nc.declare_dram_parameter("uq", [QUADS, P, 4, RANK], FP16, isOutput=False)
    x8 = nc.declare_dram_parameter("x8", [QUADS, P, 4, BS], FP8, isOutput=False)
    wctx = nc.declare_dram_parameter("wctx", [P, KC_CTX, UXW], FP16, isOutput=False)
    vt = nc.declare_dram_parameter("vt", [MG, P, RT, MW], FP16, isOutput=False)
    consts = nc.declare_dram_parameter("consts", [P, 2 * RT + MT], FP32, isOutput=False)
    outT = nc.declare_dram_parameter("outT", [OG, P, 2, BS], FP16, isOutput=True)

    with tile.TileContext(nc) as tc:
        with (
            tc.tile_pool(name="cpool", bufs=1) as cpool,
            tc.tile_pool(name="small", bufs=1) as small,
            tc.tile_pool(name="stream", bufs=1) as stream,
            tc.tile_pool(name="acts", bufs=1) as acts,
            tc.tile_pool(name="ostage", bufs=10) as ostage,
            tc.tile_pool(name="pwork", bufs=6, space="PSUM") as pwork,
            tc.tile_pool(name="pt", bufs=1, space="PSUM") as pt,
        ):
            # ---- constants + small weights ----
            c_sb = cpool.tile([P, 2 * RT + MT], FP32, tag="consts")
            nc.sync.dma_start(c_sb[:], consts[:])
            s2_sb = c_sb[:, 0:RT]
            b2_sb = c_sb[:, RT:2 * RT]
            bias_sb = c_sb[:, 2 * RT:]

            wctx_sb = small.tile([P, KC_CTX, UXW], FP16, tag="wctx")

            # ---- PE warm-up: the HAM clock gate keeps the PE at 1.2 GHz
            # until it has been busy ~3.4us. Junk matmuls on a memset tile
            # during the (otherwise idle) DMA prologue flip it to 2.4 GHz
            # before the real stream begins.
            junk = acts.tile([P, BS], FP16, tag="junk")
            nc.gpsimd.memset(junk[:], 0.0)
            warm_ps = pwork.tile([P, BS], FP32, tag="o", name="warm_ps")
            for _ in range(4):
                nc.tensor.matmul(
                    warm_ps[:],
                    junk[:, :P],
                    junk[:],
                    start=True,
                    stop=True,
                    skip_group_check=True,
                )

            # ---- bulk input stream, all on the sync ring: ux then vt.
            # wctx chunks ride the sync ring interleaved with the first ux
            # quads: chunk k lands just before the PE's laced chi chunk k
            # reaches it, without a separate ring getting starved.
            uq_sb = [stream.tile([P, 4, RANK], FP16, tag=f"uq{g}", name=f"uq{g}")
                     for g in range(QUADS)]
            x8_sb = [stream.tile([P, 4, BS], FP8, tag=f"x8{g}", name=f"x8{g}")
                     for g in range(QUADS)]
            for g in range(QUADS):
                if g < KC_CTX:
                    nc.sync.dma_start(wctx_sb[:, g, :], wctx[:, g, :])
                nc.sync.dma_start(uq_sb[g][:], uq[g])
                nc.sync.dma_start(x8_sb[g][:], x8[g])
            vt_sb = [small.tile([P, RT, MW], FP16, tag=f"vt{g}", name=f"vt{g}")
                     for g in range(MG)]
            for g in range(MG):
                nc.sync.dma_start(vt_sb[g][:], vt[g])

            # chi epilogue targets: sigmoid(+B) on ACT, *S on DVE. Queued
            # early; both engines are idle until the output phase.
            psum_chi = [pwork.tile([P, BS], FP32, tag="o", name=f"pchi{rt}")
                        for rt in range(RT)]
            s_chi = [acts.tile([P, BS], FP32, tag=f"schi{rt}", name=f"schi{rt}")
                     for rt in range(RT)]

            def emit_chi_chunk(k):
                # one ctx@W contraction chunk for both rank tiles; chunk 0
                # opens the accumulation, chunk 3 closes it and chains the
                # sigmoid + S-multiply.
                for rt in range(RT):
                    nc.tensor.matmul(
                        psum_chi[rt][:],
                        wctx_sb[:, k, rt * P:(rt + 1) * P],
                        wctx_sb[:, k, RANK:],
                        start=(k == 0),
                        stop=(k == KC_CTX - 1),
                        skip_group_check=True,
                    )
                if k == KC_CTX - 1:
                    for rt in range(RT):
                        nc.scalar.activation(
                            s_chi[rt][:], psum_chi[rt][:],
                            mybir.ActivationFunctionType.Sigmoid,
                            bias=b2_sb[:, rt:rt + 1], scale=1.0,
                        )
                        nc.vector.tensor_scalar_mul(
                            s_chi[rt][:], s_chi[rt][:], s2_sb[:, rt:rt + 1]
                        )

            # ---- t.T = (U.T @ xT): stream the packed u/x pairs. The PE is
            # the critical resource end-to-end (sustained load drops the
            # core clock to ~5/6 nominal), so chi's 8 matmuls are laced into
            # the first four mm1 pairs where the DMA pacing leaves ~140ns of
            # PE slack per pair, instead of running serially before mm1.
            psum_t = [pt.tile([P, BS], FP32, tag=f"t{rt}", name=f"pt{rt}")
                      for rt in range(RT)]
            for g in range(QUADS):
                if g < KC_CTX:
                    emit_chi_chunk(g)
                if g == QUADS - 1:
                    # rank-major: finish rt0's accumulation first so t'(rt0)
                    # overlaps the closing rt1 matmuls.
                    for rt in range(RT):
                        for j in range(4):
                            nc.tensor.matmul(
                                psum_t[rt][:],
                                uq_sb[g][:, j, rt * P:(rt + 1) * P],
                                x8_sb[g][:, j, :],
                                start=False,
                                stop=(j == 3),
                                skip_group_check=True,
                            )
                else:
                    for j in range(4):
                        k = 4 * g + j
                        for rt in range(RT):
                            nc.tensor.matmul(
                                psum_t[rt][:],
                                uq_sb[g][:, j, rt * P:(rt + 1) * P],
                                x8_sb[g][:, j, :],
                                start=(k == 0),
                                stop=False,
                                skip_group_check=True,
                            )

            # t' = t * (S*chi), cast to fp16 for mm2
            t_sb = [acts.tile([P, BS], FP16, tag=f"t{rt}", name=f"t_sb{rt}")
                    for rt in range(RT)]
            for rt in range(RT):
                nc.vector.tensor_mul(t_sb[rt][:], psum_t[rt][:], s_chi[rt][:])

            # ---- out.T = relu(V @ t.T + 2*bias): one PSUM bank per
            # 128-unit tile rotating through pwork; ACT evacuates even
            # tiles, DVE odd ones; one out DMA per evacuated pair.
            for og in range(OG):
                osb = ostage.tile([P, 2, BS], FP16, tag="osb", name=f"osb{og}")
                for j in range(2):
                    m = 2 * og + j
                    vg, off = divmod(m, MT // MG)
                    po = pwork.tile([P, BS], FP32, tag="o", name=f"po{m}")
                    for c in range(RT):
                        nc.tensor.matmul(
                            po[:],
                            vt_sb[vg][:, c, off * P:(off + 1) * P],
                            t_sb[c][:],
                            start=(c == 0),
                            stop=(c == RT - 1),
                            skip_group_check=True,
                        )
                    if j == 0:
                        nc.scalar.activation(
                            osb[:, j, :], po[:],
                            mybir.ActivationFunctionType.Relu,
                            bias=bias_sb[:, m:m + 1], scale=1.0,
                        )
                    else:
                        nc.vector.tensor_scalar(
                            osb[:, j, :], po[:],
                            bias_sb[:, m:m + 1], 0.0,
                            op0=mybir.AluOpType.add, op1=mybir.AluOpType.max,
                        )
                nc.sync.dma_start(outT[og], osb[:])

    nc.finalize()
    return nc


_NC_CACHE = {}


def _get_nc():
    if "nc" not in _NC_CACHE:
        _NC_CACHE["nc"] = _build_nc()
    return _NC_CACHE["nc"]


def _prepare_in_maps(inputs, context, U, S, V, W, B, bias):
    import ml_dtypes
    f16 = np.float16
    f8 = ml_dtypes.float8_e3m4
    xT = np.ascontiguousarray(np.asarray(inputs, dtype=np.float32).T).astype(f8)
    ctxT = np.ascontiguousarray(np.asarray(context, dtype=np.float32).T).astype(f16)

    # U quads, shared: [QUADS, P, 4, RANK]
    u4 = np.asarray(U, dtype=np.float32).astype(f16) \
        .reshape(QUADS, 4, P, RANK).transpose(0, 2, 1, 3)
    # W chunks, shared: [P, KC_CTX, RANK]
    w3 = np.asarray(W, dtype=np.float32).astype(f16) \
        .reshape(KC_CTX, P, RANK).transpose(1, 0, 2)
    # V.T groups, shared: [MG, P, RT, MW]
    vt4 = np.ascontiguousarray(np.asarray(V, dtype=np.float32).T).astype(f16) \
        .reshape(RT, P, MG, MW).transpose(2, 1, 0, 3)
    vt4 = np.ascontiguousarray(vt4)

    S2 = np.asarray(S, dtype=np.float32).reshape(RT, P).T
    B2 = np.asarray(B, dtype=np.float32).reshape(RT, P).T
    bias2 = (2.0 * np.asarray(bias, dtype=np.float32)).reshape(MT, P).T
    consts = np.ascontiguousarray(np.concatenate([S2, B2, bias2], axis=1))

    in_maps = []
    for c in range(N_CORES):
        sl = slice(c * BS, (c + 1) * BS)
        x4 = np.ascontiguousarray(
            xT[:, sl].reshape(QUADS, 4, P, BS).transpose(0, 2, 1, 3))
        ctx3 = ctxT[:, sl].reshape(KC_CTX, P, BS).transpose(1, 0, 2)
        wctx = np.empty((P, KC_CTX, UXW), dtype=f16)
        wctx[..., :RANK] = w3
        wctx[..., RANK:] = ctx3
        in_maps.append({
            "uq": np.ascontiguousarray(u4),
            "x8": x4,
            "wctx": wctx,
            "vt": vt4,
            "consts": consts,
        })
    return in_maps


def _gather_out(results):
    out = np.empty((B_SZ, UNITS), dtype=np.float32)
    for c in range(N_CORES):
        oT = np.asarray(results[c]["outT"])  # [OG, P, 2, BS] fp16
        out[c * BS:(c + 1) * BS, :] = (
            oT.transpose(3, 0, 2, 1).reshape(BS, UNITS).astype(np.float32)
        )
    return out


def kernel(inputs, context, U, S, V, W, B, bias):
    in_maps = _prepare_in_maps(inputs, context, U, S, V, W, B, bias)
    nc = _get_nc()
    res = run_bass_kernel_spmd(nc, in_maps, list(range(N_CORES)))
    return _gather_out(res.results)
